# revision 1
# baseline (speedup 1.0000x reference)
"""Trainium2 Bass kernel for nn_MultiHeadSelfAttentionModule_6193342840934.

Reference math (per batch row b of x[B,S,D]):
    xn  = LayerNorm(x) * ln_g + ln_b
    Q/K/V = xn @ w{q,k,v} + b{q,k,v}   (heads H=16, dk=64)
    scores = Q K^T / sqrt(dk) + rel_bias[h]          (S=32)
    out = x + softmax(scores) @ V @ wo + bo

Distribution: pure data-parallel over the batch dim, 2048/8 = 256 batches
(8192 tokens) per NeuronCore. Weights are replicated to every core.

Per-core kernel layout strategy:
  - tokens processed in 512-token super-tiles (16 per core)
  - LayerNorm token-major via bn_stats; ln_g/ln_b folded into the weights
    host-side (exact): wq' = ln_g*wq, bq' = bq + ln_b@wq, etc.
  - xn transposed to d-major (PE transpose) for the projections; all matmul
    inputs cast to bf16 (fp32 matmul is 4x slower on TRN2's PE)
  - scores computed TRANSPOSED (scoresT[k,q] = K'^T Q) so that softmax's
    denominator can be computed with a block-diagonal-ones matmul over the
    partition dim and the attention application needs no transpose of the
    attention matrix. 1/sqrt(dk) folded into K evacuation, rel_bias added
    via an identity-matmul accumulated into the same PSUM region.
  - IMPORTANT HW CONSTRAINT (found empirically on this runtime): consecutive
    matmuls whose operands sit at different base partitions (different PE
    row groups) crash the device. Every matmul therefore uses the full
    128-partition contraction; sub-128 contractions are expressed by
    zero-padding one operand (exact in fp: 0*x = 0), and only the PSUM
    column position varies via tile_position.
  - softmax without max-subtraction (scores are O(10) here; exp is safe in
    fp32) to save two DVE passes; denominators via ones-matmul (replicated
    across the 32 k-partitions), reciprocal_approx_fast, one TT multiply.
  - ctx computed d-major, output projection token-major, residual added by
    the PSUM-evacuating tensor_add.
"""

import numpy as np
import ml_dtypes

import concourse.bass as bass
import concourse.tile as tile
import concourse.mybir as mybir
from concourse.vector_clock import ScopedClock

dt = mybir.dt
AF = mybir.ActivationFunctionType

B, S, D, H = 2048, 32, 1024, 16
DK = D // H          # 64
EPS = 1e-5
N_CORES = 8
BPC = B // N_CORES   # 256 batches per core
TPC = BPC * S        # 8192 tokens per core
ST = 512             # tokens per super-tile
NSUB = ST // 128     # 4 sub-tiles of 128 tokens
NSUP = TPC // ST     # 16 super-tiles
NCH = D // 128       # 8 d-chunks

BF16 = ml_dtypes.bfloat16


class SplitDrainTileContext(tile.TileContext):
    """This container's walrus build rejects >1 sync-wait on a Drain
    instruction; split the tail drain's waits across standalone NOPs."""

    def _drain_and_barrier(self, tick_clock, wait_clock):
        drain_inst = self.nc.sync.drain()
        wait_clock.add_sem_waits(
            drain_inst.ins, ScopedClock({None: tick_clock.global_clock})
        )
        si = drain_inst.ins.sync_info
        waits = list(si.on_wait or []) if si is not None else []
        if len(waits) > 1:
            drain_inst.ins.sync_info.on_wait = waits[:1]
            for w in waits[1:]:
                nop = self.nc.sync.nop(hint="drain_split_wait", nofuse=True)
                nop.ins.sync_info = mybir.SyncInfo(on_wait=[w], on_update=[])
        self.nc.all_engine_barrier()
        assert self.sems is not None
        popped = self.nc._tile_sem_poison_stack.pop()
        assert popped is self._sem_poison
        self.nc.clear_and_free_semaphores(list(self.sems.allocated().values()))
        self.nc.all_engine_barrier()


def _split_excess_waits(nc: bass.Bass):
    """This container's walrus accepts at most 1 sync-wait per instruction
    (2 for EventSemaphore), but this tile version assigns up to 4. Move
    excess waits onto injected same-engine NoOps right before the
    instruction — engine streams are in-order, so this is equivalent."""
    for f in nc.m.functions:
        for bb in f.blocks:
            insts = list(bb.instructions)
            out = []
            changed = False
            for inst in insts:
                si = inst.sync_info
                cap = 2 if inst.opcode == "EventSemaphore" else 1
                waits = list(si.on_wait) if si is not None and si.on_wait else []
                if len(waits) > cap:
                    changed = True
                    for w in waits[cap:]:
                        nop = mybir.InstNoOp(
                            name=nc.get_next_instruction_name(),
                            engine=inst.engine,
                            sync_info=mybir.SyncInfo(on_wait=[w], on_update=[]),
                            bass_nofuse=True,
                        )
                        out.append(nop)
                    inst.sync_info = mybir.SyncInfo(
                        on_wait=waits[:cap], on_update=list(si.on_update or [])
                    )
                out.append(inst)
            if changed:
                bb.instructions = out


def build_nc(repeat: int = 1, split_waits: bool = True) -> bass.Bass:
    """Build the per-core Bass module. repeat>1 wraps the body in a hardware
    loop (used only for benchmarking slope timing). split_waits applies the
    walrus 1-wait-per-instruction workaround (disable for CoreSim runs)."""
    nc = bass.Bass("TRN2", target_bir_lowering=False, debug=False, num_devices=1)

    f32 = dt.float32
    bf16 = dt.bfloat16

    x_d = nc.dram_tensor("x", [TPC, D], f32, kind="ExternalInput").ap()
    y_d = nc.dram_tensor("y", [TPC, D], f32, kind="ExternalOutput").ap()
    wq_d = nc.dram_tensor("wqs", [D, D], bf16, kind="ExternalInput").ap()
    wk_d = nc.dram_tensor("wks", [D, D], bf16, kind="ExternalInput").ap()
    wv_d = nc.dram_tensor("wvs", [D, D], bf16, kind="ExternalInput").ap()
    wo_d = nc.dram_tensor("wos", [D, D], bf16, kind="ExternalInput").ap()
    bq_d = nc.dram_tensor("bq_eff", [128, NCH], f32, kind="ExternalInput").ap()
    bk_d = nc.dram_tensor("bk_eff", [128, NCH], f32, kind="ExternalInput").ap()
    # relpad[j, h*32+q] = rel_bias[h, q, j] for j<32, 0 for j>=32
    rel_d = nc.dram_tensor("relpad", [128, H * 32], bf16, kind="ExternalInput").ap()
    id_d = nc.dram_tensor("ident", [128, 128], bf16, kind="ExternalInput").ap()
    # id4pad[j, p] = (j == p % 32) for j<32, 0 for j>=32
    id4_d = nc.dram_tensor("id4pad", [128, 128], bf16, kind="ExternalInput").ap()
    # bdones[(b,k), (b',m)] = (b == b')  (32-block diagonal of ones)
    bdon_d = nc.dram_tensor("bdones", [128, 128], bf16, kind="ExternalInput").ap()

    with SplitDrainTileContext(nc) as tc:
        with (
            tc.tile_pool(name="consts", bufs=1) as consts,
            tc.tile_pool(name="xin", bufs=6) as xin_pool,
            tc.tile_pool(name="small", bufs=8) as small,
            tc.tile_pool(name="xn0", bufs=2) as xn0_pool,
            tc.tile_pool(name="xnT", bufs=2) as xnT_pool,
            tc.tile_pool(name="qk", bufs=2) as qk_pool,
            tc.tile_pool(name="vsb", bufs=2) as v_pool,
            tc.tile_pool(name="attn", bufs=2) as attn_pool,
            tc.tile_pool(name="ctx", bufs=2) as ctx_pool,
            tc.tile_pool(name="osb", bufs=2) as out_pool,
            tc.tile_pool(name="ps_proj", bufs=3, space="PSUM") as ps_proj,
            tc.tile_pool(name="ps_attn", bufs=2, space="PSUM") as ps_attn,
            tc.tile_pool(name="ps_ctx", bufs=2, space="PSUM") as ps_ctx,
            tc.tile_pool(name="ps_xp", bufs=1, space="PSUM") as ps_xp,
        ):
            # -- resident constants -------------------------------------------
            wq_s = consts.tile([128, NCH, D], bf16)
            wk_s = consts.tile([128, NCH, D], bf16)
            wv_s = consts.tile([128, NCH, D], bf16)
            wo_s = consts.tile([128, NCH, D], bf16)
            for wsb, wd in ((wq_s, wq_d), (wk_s, wk_d), (wv_s, wv_d), (wo_s, wo_d)):
                nc.sync.dma_start(wsb, wd.rearrange("(c p) n -> p c n", p=128))
            bq_s = consts.tile([128, NCH], f32)
            nc.sync.dma_start(bq_s, bq_d)
            bk_s = consts.tile([128, NCH], f32)
            nc.sync.dma_start(bk_s, bk_d)
            rel_s = consts.tile([128, H * 32], bf16)
            nc.sync.dma_start(rel_s, rel_d)
            id_s = consts.tile([128, 128], bf16)
            nc.sync.dma_start(id_s, id_d)
            id4_s = consts.tile([128, 128], bf16)
            nc.sync.dma_start(id4_s, id4_d)
            bdon_s = consts.tile([128, 128], bf16)
            nc.sync.dma_start(bdon_s, bdon_d)
            eps_s = consts.tile([128, 1], f32)
            nc.vector.memset(eps_s, EPS)

            # per-super-tile prelude state (xts list + xnT tile), filled by
            # prelude() which is emitted EARLY (pipelined one super-tile ahead)
            state: dict = {}

            def prelude(sup: int, s: int):
                t0 = sup * ST
                if s == 0:
                    xnT = xnT_pool.tile([128, NCH, ST], bf16, tag="xnT")
                    state[sup] = ([], xnT)
                xts, xnT = state[sup]
                row = t0 + s * 128
                xt = xin_pool.tile([128, D], f32, tag="x")
                nc.sync.dma_start(xt, x_d[row : row + 128, :])
                xts.append(xt)
                st6 = small.tile([128, 2, 6], f32, tag="st6")
                nc.vector.bn_stats(st6[:, 0, :], xt[:, 0:512])
                nc.vector.bn_stats(st6[:, 1, :], xt[:, 512:1024])
                mv = small.tile([128, 2], f32, tag="mv")
                nc.vector.bn_aggr(mv, st6)
                sd = small.tile([128, 1], f32, tag="sd")
                nc.scalar.activation(sd, mv[:, 1:2], AF.Sqrt, bias=eps_s[:])
                rsig = small.tile([128, 1], f32, tag="rsig")
                nc.vector.reciprocal(rsig, sd)
                nmu = small.tile([128, 1], f32, tag="nmu")
                nc.vector.tensor_mul(nmu, mv[:, 0:1], rsig)
                nmr = small.tile([128, 1], f32, tag="nmr")
                nc.vector.tensor_scalar_mul(nmr, nmu, -1.0)
                xn0 = xn0_pool.tile([128, D], bf16, tag="xn0")
                nc.scalar.activation(xn0, xt, AF.Identity, bias=nmr[:], scale=rsig[:])
                xp = ps_xp.tile([128, NCH, 128], bf16, tag="xp")
                for c in range(NCH):
                    nc.tensor.transpose(xp[:, c, :], xn0[:, c * 128 : (c + 1) * 128], id_s)
                nc.vector.tensor_copy(xnT[:, :, s * 128 : (s + 1) * 128], xp)

            def super_tile(sup: int):
                t0 = sup * ST
                xts, xnT = state[sup]

                # ---- Q, K projections (d-major, bf16, N=512) ----------------
                # K lands in a zero-padded per-head layout kpad[128, H, ST]:
                # head h occupies partitions (h%2)*64..+64 of slot h, the other
                # 64 partitions stay zero, so score matmuls can contract over
                # the full 128 partitions (see HW constraint above).
                qs = qk_pool.tile([128, NCH, ST], bf16, tag="q")
                kpad = qk_pool.tile([128, H, ST], bf16, tag="kpad")
                # gpsimd is otherwise idle; keep the zero-padding off DVE
                nc.gpsimd.memset(kpad, 0.0)
                for c in range(NCH):
                    ps = ps_proj.tile([128, 512], f32, tag="proj")
                    for ci in range(NCH):
                        nc.tensor.matmul(
                            ps,
                            lhsT=wq_s[:, ci, c * 128 : (c + 1) * 128],
                            rhs=xnT[:, ci, :],
                            start=(ci == 0),
                            stop=(ci == NCH - 1),
                        )
                    nc.scalar.activation(
                        qs[:, c, :], ps, AF.Identity, bias=bq_s[:, c : c + 1], scale=1.0
                    )
                for c in range(NCH):
                    ps = ps_proj.tile([128, 512], f32, tag="proj")
                    for ci in range(NCH):
                        nc.tensor.matmul(
                            ps,
                            lhsT=wk_s[:, ci, c * 128 : (c + 1) * 128],
                            rhs=xnT[:, ci, :],
                            start=(ci == 0),
                            stop=(ci == NCH - 1),
                        )
                    nc.scalar.activation(
                        kpad[0:64, 2 * c, :], ps[0:64, :], AF.Identity,
                        bias=bk_s[0:64, c : c + 1], scale=0.125,
                    )
                    nc.scalar.activation(
                        kpad[64:128, 2 * c + 1, :], ps[64:128, :], AF.Identity,
                        bias=bk_s[64:128, c : c + 1], scale=0.125,
                    )

                # ---- V projection (token-major, bf16) -----------------------
                vs = v_pool.tile([128, NSUB, D], bf16, tag="v")
                for s in range(NSUB):
                    for half in range(2):
                        ps = ps_proj.tile([128, 512], f32, tag="proj")
                        for ci in range(NCH):
                            nc.tensor.matmul(
                                ps,
                                lhsT=xnT[:, ci, s * 128 : (s + 1) * 128],
                                rhs=wv_s[:, ci, half * 512 : (half + 1) * 512],
                                start=(ci == 0),
                                stop=(ci == NCH - 1),
                            )
                        nc.vector.tensor_copy(vs[:, s, half * 512 : (half + 1) * 512], ps)

                # ---- attention + output projection, per sub-tile ------------
                for s in range(NSUB):
                    # software pipeline: emit the next super-tile's LN+transpose
                    # preludes here so their DMA/DVE/ACT/PE-transpose chain
                    # fills the PE idle window at the super-tile boundary
                    if sup + 1 < NSUP:
                        if s == 0:
                            prelude(sup + 1, 0)
                            prelude(sup + 1, 1)
                        elif s == 2:
                            prelude(sup + 1, 2)
                            prelude(sup + 1, 3)
                    # scoresT[(b,k), (h,q)] = K'^T Q + rel_biasT  (PSUM bank)
                    # NB: skip_group_check — the sim's coarse PSUM zero-region
                    # bookkeeping can't express "one full-region start, many
                    # sub-block accumulates"; on HW this is per-element
                    # has_written and PE executes in program order.
                    sc = ps_attn.tile([128, H * 32], f32, tag="attn")
                    nc.tensor.matmul(
                        sc, lhsT=id4_s, rhs=rel_s, start=True, stop=False,
                        skip_group_check=True,
                    )
                    for h in range(H):
                        for b in range(4):
                            tok = slice(s * 128 + b * 32, s * 128 + (b + 1) * 32)
                            nc.tensor.matmul(
                                sc[b * 32 : (b + 1) * 32, h * 32 : (h + 1) * 32],
                                lhsT=kpad[:, h, tok],
                                rhs=qs[:, h // 2, tok],
                                start=False,
                                stop=(h == H - 1),
                                tile_position=(0, b * 32),
                                skip_group_check=True,
                            )
                    at_u = attn_pool.tile([128, H * 32], bf16, tag="atu")
                    nc.scalar.activation(at_u, sc, AF.Exp)
                    # per-batch-block softmax denominators, replicated across
                    # each 32-row block by the block-diagonal ones matmul
                    dn = ps_attn.tile([128, H * 32], f32, tag="attn")
                    nc.tensor.matmul(dn, lhsT=bdon_s, rhs=at_u, start=True, stop=True)
                    # 1/denom via exp(-ln(x)) on ACT: this walrus build rejects
                    # the custom-DVE fast-reciprocal ISA op, and the native DVE
                    # reciprocal is ~8 cyc/elem. LUT rel-err ~1e-4 is fine at
                    # bf16 noise levels.
                    lnd = attn_pool.tile([128, H * 32], f32, tag="lnd")
                    nc.scalar.activation(lnd, dn, AF.Ln)
                    rc = attn_pool.tile([128, H * 32], f32, tag="rc")
                    nc.scalar.activation(rc, lnd, AF.Exp, scale=-1.0)
                    at = attn_pool.tile([128, H * 32], bf16, tag="at")
                    nc.vector.tensor_mul(at, at_u, rc)

                    # block-diagonalize attnT per head: at_bd[(b,k), h, (b,q)]
                    # nonzero only for matching b, so AV can contract over the
                    # full 128 token partitions.
                    at_bd = attn_pool.tile([128, H, 128], bf16, tag="at_bd")
                    nc.gpsimd.memset(at_bd, 0.0)
                    atv = at.rearrange("p (h q) -> p h q", h=H)
                    for b in range(4):
                        nc.vector.tensor_copy(
                            at_bd[b * 32 : (b + 1) * 32, :, b * 32 : (b + 1) * 32],
                            atv[b * 32 : (b + 1) * 32, :, :],
                        )

                    # ctxT[(h,dv), t] d-major: one matmul per head over all 4
                    # batches at once (cross-batch terms killed by at_bd zeros)
                    ctxT = ctx_pool.tile([128, NCH, 128], bf16, tag="ctxT")
                    for g in range(2):
                        cps = ps_ctx.tile([128, 4, 128], f32, tag="ctx")
                        for h in range(g * 8, g * 8 + 8):
                            pb = (h % 2) * 64
                            nc.tensor.matmul(
                                cps[pb : pb + 64, (h // 2) % 4, :],
                                lhsT=vs[:, s, h * 64 : (h + 1) * 64],
                                rhs=at_bd[:, h, :],
                                start=True,
                                stop=True,
                                tile_position=(0, pb),
                            )
                        nc.scalar.activation(ctxT[:, g * 4 : (g + 1) * 4, :], cps, AF.Copy)

                    # out = x + ctx @ wo   (token-major)
                    outsb = out_pool.tile([128, D], f32, tag="osb")
                    for half in range(2):
                        ps = ps_proj.tile([128, 512], f32, tag="proj")
                        for c in range(NCH):
                            nc.tensor.matmul(
                                ps,
                                lhsT=ctxT[:, c, :],
                                rhs=wo_s[:, c, half * 512 : (half + 1) * 512],
                                start=(c == 0),
                                stop=(c == NCH - 1),
                            )
                        nc.vector.tensor_add(
                            outsb[:, half * 512 : (half + 1) * 512],
                            xts[s][:, half * 512 : (half + 1) * 512],
                            ps,
                        )
                    row = t0 + s * 128
                    nc.sync.dma_start(y_d[row : row + 128, :], outsb)

            def run_all():
                for s in range(NSUB):
                    prelude(0, s)
                for sup in range(NSUP):
                    super_tile(sup)

            if repeat > 1:
                with tc.For_i(0, repeat, 1):
                    run_all()
            else:
                run_all()

    if split_waits:
        _split_excess_waits(nc)
    return nc


def _host_constants(ln_g, ln_b, wq, bq, wk, bk, wv, bv, wo, bo, rel_bias):
    """Exact host-side weight transforms (fold LN affine + 1/sqrt(dk))."""
    f32 = np.float32
    g = ln_g.astype(f32)
    b = ln_b.astype(f32)
    wq = wq.astype(f32)
    wk = wk.astype(f32)
    wv = wv.astype(f32)
    wo = wo.astype(f32)
    wqs = (g[:, None] * wq).astype(BF16)
    wks = (g[:, None] * wk).astype(BF16)
    wvs = (g[:, None] * wv).astype(BF16)
    wos = wo.astype(BF16)
    bq_eff = (bq.astype(f32) + b @ wq).reshape(NCH, 128).T.copy()          # [128, NCH]
    bk_eff = ((bk.astype(f32) + b @ wk) * 0.125).reshape(NCH, 128).T.copy()
    # relpad[j, h*32+q] = rel_bias[h, q, j] for j<32, zero-padded to 128 rows
    relpad = np.zeros((128, H * 32), dtype=f32)
    relpad[:32] = rel_bias.astype(f32).transpose(2, 0, 1).reshape(32, H * 32)
    ident = np.eye(128, dtype=f32).astype(BF16)
    id4pad = np.zeros((128, 128), dtype=f32)
    id4pad[:32] = np.tile(np.eye(32, dtype=f32), (1, 4))
    bdones = np.kron(np.eye(4, dtype=f32), np.ones((32, 32), dtype=f32))
    # bv/bo/ln_b contributions that survive softmax-normalization exactly:
    # out += ((ln_b@wv + bv) @ wo + bo). Zero for this problem's fills.
    c0 = (b @ wv + bv.astype(f32)) @ wo + bo.astype(f32)
    return dict(
        wqs=wqs, wks=wks, wvs=wvs, wos=wos,
        bq_eff=np.ascontiguousarray(bq_eff), bk_eff=np.ascontiguousarray(bk_eff),
        relpad=relpad.astype(BF16), ident=ident,
        id4pad=id4pad.astype(BF16), bdones=bdones.astype(BF16),
    ), c0


_BUILT = {}


def _get_nc(repeat: int = 1):
    if repeat not in _BUILT:
        _BUILT[repeat] = build_nc(repeat)
    return _BUILT[repeat]


def make_in_maps(inputs: dict, consts: dict) -> list:
    x = np.asarray(inputs["x"], dtype=np.float32).reshape(B * S, D)
    in_maps = []
    for c in range(N_CORES):
        m = dict(consts)
        m["x"] = np.ascontiguousarray(x[c * TPC : (c + 1) * TPC])
        in_maps.append(m)
    return in_maps


def kernel(**inputs) -> np.ndarray:
    from concourse.bass_utils import run_bass_kernel_spmd

    consts, c0 = _host_constants(
        inputs["ln_g"], inputs["ln_b"], inputs["wq"], inputs["bq"],
        inputs["wk"], inputs["bk"], inputs["wv"], inputs["bv"],
        inputs["wo"], inputs["bo"], inputs["rel_bias"],
    )
    nc = _get_nc(1)
    in_maps = make_in_maps(inputs, consts)
    res = run_bass_kernel_spmd(nc, in_maps, core_ids=list(range(N_CORES)), trace=False)
    out = np.concatenate([res.results[c]["y"] for c in range(N_CORES)], axis=0)
    out = out.reshape(B, S, D)
    if np.any(c0 != 0.0):
        out = out + c0.astype(np.float32)
    return out



# revision 10
# speedup vs baseline: 1.1268x; 1.1268x over previous
"""Trainium2 Bass kernel for nn_MultiHeadSelfAttentionModule_6193342840934.

Reference math (per batch row b of x[B,S,D]):
    xn  = LayerNorm(x) * ln_g + ln_b
    Q/K/V = xn @ w{q,k,v} + b{q,k,v}   (heads H=16, dk=64)
    scores = Q K^T / sqrt(dk) + rel_bias[h]          (S=32)
    out = x + softmax(scores) @ V @ wo + bo

Distribution: pure data-parallel over the batch dim, 2048/8 = 256 batches
(8192 tokens) per NeuronCore. Weights are replicated to every core.

Per-core kernel layout strategy:
  - tokens processed in 512-token super-tiles (16 per core)
  - LayerNorm token-major via bn_stats; ln_g/ln_b folded into the weights
    host-side (exact): wq' = ln_g*wq, bq' = bq + ln_b@wq, etc.
  - xn transposed to d-major (PE transpose) for the projections; all matmul
    inputs cast to bf16 (fp32 matmul is 4x slower on TRN2's PE)
  - scores computed TRANSPOSED (scoresT[k,q] = K'^T Q) so that softmax's
    denominator can be computed with a block-diagonal-ones matmul over the
    partition dim and the attention application needs no transpose of the
    attention matrix. 1/sqrt(dk) folded into K evacuation, rel_bias added
    via an identity-matmul accumulated into the same PSUM region.
  - IMPORTANT HW CONSTRAINT (found empirically on this runtime): consecutive
    matmuls whose operands sit at different base partitions (different PE
    row groups) crash the device. Every matmul therefore uses the full
    128-partition contraction; sub-128 contractions are expressed by
    zero-padding one operand (exact in fp: 0*x = 0), and only the PSUM
    column position varies via tile_position.
  - softmax without max-subtraction (scores are O(10) here; exp is safe in
    fp32) to save two DVE passes; denominators via ones-matmul (replicated
    across the 32 k-partitions), reciprocal_approx_fast, one TT multiply.
  - ctx computed d-major, output projection token-major, residual added by
    the PSUM-evacuating tensor_add.
"""

import numpy as np
import ml_dtypes

import concourse.bass as bass
import concourse.tile as tile
import concourse.mybir as mybir
from concourse.vector_clock import ScopedClock

dt = mybir.dt
AF = mybir.ActivationFunctionType
PM = mybir.MatmulPerfMode

B, S, D, H = 2048, 32, 1024, 16
DK = D // H          # 64
EPS = 1e-5
N_CORES = 8
BPC = B // N_CORES   # 256 batches per core
TPC = BPC * S        # 8192 tokens per core
ST = 512             # tokens per super-tile
NSUB = ST // 128     # 4 sub-tiles of 128 tokens
NSUP = TPC // ST     # 16 super-tiles
NCH = D // 128       # 8 d-chunks

BF16 = ml_dtypes.bfloat16
F8 = ml_dtypes.float8_e4m3


class SplitDrainTileContext(tile.TileContext):
    """This container's walrus build rejects >1 sync-wait on a Drain
    instruction; split the tail drain's waits across standalone NOPs."""

    def _drain_and_barrier(self, tick_clock, wait_clock):
        drain_inst = self.nc.sync.drain()
        wait_clock.add_sem_waits(
            drain_inst.ins, ScopedClock({None: tick_clock.global_clock})
        )
        si = drain_inst.ins.sync_info
        waits = list(si.on_wait or []) if si is not None else []
        if len(waits) > 1:
            drain_inst.ins.sync_info.on_wait = waits[:1]
            for w in waits[1:]:
                nop = self.nc.sync.nop(hint="drain_split_wait", nofuse=True)
                nop.ins.sync_info = mybir.SyncInfo(on_wait=[w], on_update=[])
        self.nc.all_engine_barrier()
        assert self.sems is not None
        popped = self.nc._tile_sem_poison_stack.pop()
        assert popped is self._sem_poison
        self.nc.clear_and_free_semaphores(list(self.sems.allocated().values()))
        self.nc.all_engine_barrier()


def _split_excess_waits(nc: bass.Bass):
    """This container's walrus accepts at most 1 sync-wait per instruction
    (2 for EventSemaphore), but this tile version assigns up to 4. Move
    excess waits onto injected same-engine NoOps right before the
    instruction — engine streams are in-order, so this is equivalent."""
    for f in nc.m.functions:
        for bb in f.blocks:
            insts = list(bb.instructions)
            out = []
            changed = False
            for inst in insts:
                si = inst.sync_info
                cap = 2 if inst.opcode == "EventSemaphore" else 1
                waits = list(si.on_wait) if si is not None and si.on_wait else []
                if len(waits) > cap:
                    changed = True
                    for w in waits[cap:]:
                        nop = mybir.InstNoOp(
                            name=nc.get_next_instruction_name(),
                            engine=inst.engine,
                            sync_info=mybir.SyncInfo(on_wait=[w], on_update=[]),
                            bass_nofuse=True,
                        )
                        out.append(nop)
                    inst.sync_info = mybir.SyncInfo(
                        on_wait=waits[:cap], on_update=list(si.on_update or [])
                    )
                out.append(inst)
            if changed:
                bb.instructions = out


def build_nc(repeat: int = 1, split_waits: bool = True) -> bass.Bass:
    """Build the per-core Bass module. repeat>1 wraps the body in a hardware
    loop (used only for benchmarking slope timing). split_waits applies the
    walrus 1-wait-per-instruction workaround (disable for CoreSim runs)."""
    nc = bass.Bass("TRN2", target_bir_lowering=False, debug=False, num_devices=1)

    f32 = dt.float32
    bf16 = dt.bfloat16
    f8 = dt.float8e4

    x_d = nc.dram_tensor("x", [TPC, D], f32, kind="ExternalInput").ap()
    y_d = nc.dram_tensor("y", [TPC, D], f32, kind="ExternalOutput").ap()
    wq_d = nc.dram_tensor("wqs", [D, D], f8, kind="ExternalInput").ap()
    wk_d = nc.dram_tensor("wks", [D, D], f8, kind="ExternalInput").ap()
    wv_d = nc.dram_tensor("wvs", [D, D], f8, kind="ExternalInput").ap()
    wo_d = nc.dram_tensor("wos", [D, D], f8, kind="ExternalInput").ap()
    bq_d = nc.dram_tensor("bq_eff", [128, NCH], f32, kind="ExternalInput").ap()
    bk_d = nc.dram_tensor("bk_eff", [128, NCH], f32, kind="ExternalInput").ap()
    # relpad[j, h*32+q] = rel_bias[h, q, j] for j<32, 0 for j>=32
    rel_d = nc.dram_tensor("relpad", [128, H * 32], bf16, kind="ExternalInput").ap()
    id_d = nc.dram_tensor("ident", [128, 128], bf16, kind="ExternalInput").ap()
    # id4pad[j, p] = (j == p % 32) for j<32, 0 for j>=32
    id4_d = nc.dram_tensor("id4pad", [128, 128], bf16, kind="ExternalInput").ap()
    # bdones[(b,k), (b',m)] = (b == b')  (32-block diagonal of ones)
    bdon_d = nc.dram_tensor("bdones", [128, 128], bf16, kind="ExternalInput").ap()

    with SplitDrainTileContext(nc) as tc:
        with (
            tc.tile_pool(name="consts", bufs=1) as consts,
            tc.tile_pool(name="xin", bufs=6) as xin_pool,
            tc.tile_pool(name="small", bufs=8) as small,
            tc.tile_pool(name="xn0", bufs=2) as xn0_pool,
            tc.tile_pool(name="xnT", bufs=2) as xnT_pool,
            tc.tile_pool(name="qk", bufs=2) as qk_pool,
            tc.tile_pool(name="vsb", bufs=2) as v_pool,
            tc.tile_pool(name="attn", bufs=2) as attn_pool,
            tc.tile_pool(name="ctx", bufs=2) as ctx_pool,
            tc.tile_pool(name="osb", bufs=2) as out_pool,
            tc.tile_pool(name="ps_proj", bufs=3, space="PSUM") as ps_proj,
            tc.tile_pool(name="ps_attn", bufs=2, space="PSUM") as ps_attn,
            tc.tile_pool(name="ps_ctx", bufs=2, space="PSUM") as ps_ctx,
            tc.tile_pool(name="ps_xp", bufs=1, space="PSUM") as ps_xp,
        ):
            # -- resident constants -------------------------------------------
            wq_s = consts.tile([128, NCH, D], f8)
            wk_s = consts.tile([128, NCH, D], f8)
            wv_s = consts.tile([128, NCH, D], f8)
            wo_s = consts.tile([128, NCH, D], f8)
            for wsb, wd in ((wq_s, wq_d), (wk_s, wk_d), (wv_s, wv_d), (wo_s, wo_d)):
                nc.sync.dma_start(wsb, wd.rearrange("(c p) n -> p c n", p=128))
            bq_s = consts.tile([128, NCH], f32)
            nc.sync.dma_start(bq_s, bq_d)
            bk_s = consts.tile([128, NCH], f32)
            nc.sync.dma_start(bk_s, bk_d)
            rel_s = consts.tile([128, H * 32], bf16)
            nc.sync.dma_start(rel_s, rel_d)
            id_s = consts.tile([128, 128], bf16)
            nc.sync.dma_start(id_s, id_d)
            id4_s = consts.tile([128, 128], bf16)
            nc.sync.dma_start(id4_s, id4_d)
            bdon_s = consts.tile([128, 128], bf16)
            nc.sync.dma_start(bdon_s, bdon_d)
            eps_s = consts.tile([128, 1], f32)
            nc.vector.memset(eps_s, EPS)

            # persistent zero-padded tiles (double-buffered by hand): the
            # zero regions are written once here and never touched again —
            # evacuations only write the valid blocks, so the per-super-tile
            # gpsimd memsets of the baseline are hoisted out of the loop.
            kpads = []
            for i in range(2):
                kp = consts.tile([128, H, ST], bf16, tag=f"kpadp{i}")
                nc.gpsimd.memset(kp, 0.0)
                kpads.append(kp)
            at_bds = []
            for i in range(2):
                ab = consts.tile([128, H, 128], bf16, tag=f"atbdp{i}")
                nc.gpsimd.memset(ab, 0.0)
                at_bds.append(ab)

            # per-super-tile prelude state (xts list + xnT tile), filled by
            # prelude() which is emitted EARLY (pipelined one super-tile ahead)
            state: dict = {}

            def prelude(sup: int, s: int):
                t0 = sup * ST
                if s == 0:
                    xnT = xnT_pool.tile([128, NCH, ST], f8, tag="xnT")
                    state[sup] = ([], xnT)
                xts, xnT = state[sup]
                row = t0 + s * 128
                xt = xin_pool.tile([128, D], f32, tag="x")
                nc.sync.dma_start(xt, x_d[row : row + 128, :])
                xts.append(xt)
                st6 = small.tile([128, 2, 6], f32, tag="st6")
                nc.vector.bn_stats(st6[:, 0, :], xt[:, 0:512])
                nc.vector.bn_stats(st6[:, 1, :], xt[:, 512:1024])
                mv = small.tile([128, 2], f32, tag="mv")
                nc.vector.bn_aggr(mv, st6)
                sd = small.tile([128, 1], f32, tag="sd")
                nc.scalar.activation(sd, mv[:, 1:2], AF.Sqrt, bias=eps_s[:])
                rsig = small.tile([128, 1], f32, tag="rsig")
                nc.vector.reciprocal(rsig, sd)
                nmu = small.tile([128, 1], f32, tag="nmu")
                nc.vector.tensor_mul(nmu, mv[:, 0:1], rsig)
                nmr = small.tile([128, 1], f32, tag="nmr")
                nc.vector.tensor_scalar_mul(nmr, nmu, -1.0)
                xn0 = xn0_pool.tile([128, D], bf16, tag="xn0")
                nc.scalar.activation(xn0, xt, AF.Identity, bias=nmr[:], scale=rsig[:])
                xp = ps_xp.tile([128, NCH, 128], bf16, tag="xp")
                for c in range(NCH):
                    nc.tensor.transpose(xp[:, c, :], xn0[:, c * 128 : (c + 1) * 128], id_s)
                nc.vector.tensor_copy(xnT[:, :, s * 128 : (s + 1) * 128], xp)

            def super_tile(sup: int):
                t0 = sup * ST
                xts, xnT = state[sup]

                # ---- Q, K projections (d-major, fp8 DoubleRow, N=512) -------
                # Both operands fp8e4; DoubleRow packs two 128-d chunks per
                # matmul (256-deep contraction) for 2x PE throughput.
                # K lands in a zero-padded per-head layout kpad[128, H, ST]:
                # head h occupies partitions (h%2)*64..+64 of slot h, the other
                # 64 partitions stay zero, so score matmuls can contract over
                # the full 128 partitions (see HW constraint above).
                qs = qk_pool.tile([128, NCH, ST], bf16, tag="q")
                kpad = kpads[sup % 2]
                for c in range(NCH):
                    ps = ps_proj.tile([128, 512], f32, tag="proj")
                    for cp in range(NCH // 2):
                        nc.tensor.matmul(
                            ps,
                            lhsT=wq_s[:, 2 * cp : 2 * cp + 2, c * 128 : (c + 1) * 128],
                            rhs=xnT[:, 2 * cp : 2 * cp + 2, :],
                            start=(cp == 0),
                            stop=(cp == NCH // 2 - 1),
                            perf_mode=PM.DoubleRow,
                        )
                    nc.scalar.activation(
                        qs[:, c, :], ps, AF.Identity, bias=bq_s[:, c : c + 1], scale=1.0
                    )
                for c in range(NCH):
                    ps = ps_proj.tile([128, 512], f32, tag="proj")
                    for cp in range(NCH // 2):
                        nc.tensor.matmul(
                            ps,
                            lhsT=wk_s[:, 2 * cp : 2 * cp + 2, c * 128 : (c + 1) * 128],
                            rhs=xnT[:, 2 * cp : 2 * cp + 2, :],
                            start=(cp == 0),
                            stop=(cp == NCH // 2 - 1),
                            perf_mode=PM.DoubleRow,
                        )
                    nc.scalar.activation(
                        kpad[0:64, 2 * c, :], ps[0:64, :], AF.Identity,
                        bias=bk_s[0:64, c : c + 1], scale=0.125,
                    )
                    nc.scalar.activation(
                        kpad[64:128, 2 * c + 1, :], ps[64:128, :], AF.Identity,
                        bias=bk_s[64:128, c : c + 1], scale=0.125,
                    )

                # ---- V projection (token-major, fp8 DoubleRow) --------------
                vs = v_pool.tile([128, NSUB, D], bf16, tag="v")
                for s in range(NSUB):
                    for half in range(2):
                        ps = ps_proj.tile([128, 512], f32, tag="proj")
                        for cp in range(NCH // 2):
                            nc.tensor.matmul(
                                ps,
                                lhsT=xnT[:, 2 * cp : 2 * cp + 2, s * 128 : (s + 1) * 128],
                                rhs=wv_s[:, 2 * cp : 2 * cp + 2, half * 512 : (half + 1) * 512],
                                start=(cp == 0),
                                stop=(cp == NCH // 2 - 1),
                                perf_mode=PM.DoubleRow,
                            )
                        nc.vector.tensor_copy(vs[:, s, half * 512 : (half + 1) * 512], ps)

                # ---- attention + output projection, per sub-tile ------------
                for s in range(NSUB):
                    # software pipeline: emit the next super-tile's LN+transpose
                    # preludes here so their DMA/DVE/ACT/PE-transpose chain
                    # fills the PE idle window at the super-tile boundary
                    if sup + 1 < NSUP:
                        if s == 0:
                            prelude(sup + 1, 0)
                            prelude(sup + 1, 1)
                        elif s == 2:
                            prelude(sup + 1, 2)
                            prelude(sup + 1, 3)
                    # scoresT[(b,k), (h,q)] = K'^T Q + rel_biasT  (PSUM bank)
                    # NB: skip_group_check — the sim's coarse PSUM zero-region
                    # bookkeeping can't express "one full-region start, many
                    # sub-block accumulates"; on HW this is per-element
                    # has_written and PE executes in program order.
                    sc = ps_attn.tile([128, H * 32], f32, tag="attn")
                    nc.tensor.matmul(
                        sc, lhsT=id4_s, rhs=rel_s, start=True, stop=False,
                        skip_group_check=True,
                    )
                    for h in range(H):
                        for b in range(4):
                            tok = slice(s * 128 + b * 32, s * 128 + (b + 1) * 32)
                            nc.tensor.matmul(
                                sc[b * 32 : (b + 1) * 32, h * 32 : (h + 1) * 32],
                                lhsT=kpad[:, h, tok],
                                rhs=qs[:, h // 2, tok],
                                start=False,
                                stop=(h == H - 1),
                                tile_position=(0, b * 32),
                                skip_group_check=True,
                            )
                    at_u = attn_pool.tile([128, H * 32], bf16, tag="atu")
                    nc.scalar.activation(at_u, sc, AF.Exp)
                    # per-batch-block softmax denominators, replicated across
                    # each 32-row block by the block-diagonal ones matmul
                    dn = ps_attn.tile([128, H * 32], f32, tag="attn")
                    nc.tensor.matmul(dn, lhsT=bdon_s, rhs=at_u, start=True, stop=True)
                    # 1/denom via exp(-ln(x)) on ACT: this walrus build rejects
                    # the custom-DVE fast-reciprocal ISA op, and the native DVE
                    # reciprocal is ~8 cyc/elem. LUT rel-err ~1e-4 is fine at
                    # bf16 noise levels.
                    lnd = attn_pool.tile([128, H * 32], f32, tag="lnd")
                    nc.scalar.activation(lnd, dn, AF.Ln)
                    rc = attn_pool.tile([128, H * 32], f32, tag="rc")
                    nc.scalar.activation(rc, lnd, AF.Exp, scale=-1.0)
                    at = attn_pool.tile([128, H * 32], bf16, tag="at")
                    nc.vector.tensor_mul(at, at_u, rc)

                    # block-diagonalize attnT per head: at_bd[(b,k), h, (b,q)]
                    # nonzero only for matching b, so AV can contract over the
                    # full 128 token partitions. Zero regions are persistent.
                    at_bd = at_bds[s % 2]
                    atv = at.rearrange("p (h q) -> p h q", h=H)
                    for b in range(4):
                        nc.vector.tensor_copy(
                            at_bd[b * 32 : (b + 1) * 32, :, b * 32 : (b + 1) * 32],
                            atv[b * 32 : (b + 1) * 32, :, :],
                        )

                    # ctxT[(h,dv), t] d-major: one matmul per head over all 4
                    # batches at once (cross-batch terms killed by at_bd zeros)
                    ctxT = ctx_pool.tile([128, NCH, 128], f8, tag="ctxT")
                    for g in range(2):
                        cps = ps_ctx.tile([128, 4, 128], f32, tag="ctx")
                        for h in range(g * 8, g * 8 + 8):
                            pb = (h % 2) * 64
                            nc.tensor.matmul(
                                cps[pb : pb + 64, (h // 2) % 4, :],
                                lhsT=vs[:, s, h * 64 : (h + 1) * 64],
                                rhs=at_bd[:, h, :],
                                start=True,
                                stop=True,
                                tile_position=(0, pb),
                            )
                        nc.scalar.activation(ctxT[:, g * 4 : (g + 1) * 4, :], cps, AF.Copy)

                    # out = x + ctx @ wo   (token-major, fp8 DoubleRow)
                    outsb = out_pool.tile([128, D], f32, tag="osb")
                    for half in range(2):
                        ps = ps_proj.tile([128, 512], f32, tag="proj")
                        for cp in range(NCH // 2):
                            nc.tensor.matmul(
                                ps,
                                lhsT=ctxT[:, 2 * cp : 2 * cp + 2, :],
                                rhs=wo_s[:, 2 * cp : 2 * cp + 2, half * 512 : (half + 1) * 512],
                                start=(cp == 0),
                                stop=(cp == NCH // 2 - 1),
                                perf_mode=PM.DoubleRow,
                            )
                        nc.vector.tensor_add(
                            outsb[:, half * 512 : (half + 1) * 512],
                            xts[s][:, half * 512 : (half + 1) * 512],
                            ps,
                        )
                    row = t0 + s * 128
                    nc.sync.dma_start(y_d[row : row + 128, :], outsb)

            def run_all():
                for s in range(NSUB):
                    prelude(0, s)
                for sup in range(NSUP):
                    super_tile(sup)

            if repeat > 1:
                with tc.For_i(0, repeat, 1):
                    run_all()
            else:
                run_all()

    if split_waits:
        _split_excess_waits(nc)
    return nc


def _host_constants(ln_g, ln_b, wq, bq, wk, bk, wv, bv, wo, bo, rel_bias):
    """Exact host-side weight transforms (fold LN affine + 1/sqrt(dk))."""
    f32 = np.float32
    g = ln_g.astype(f32)
    b = ln_b.astype(f32)
    wq = wq.astype(f32)
    wk = wk.astype(f32)
    wv = wv.astype(f32)
    wo = wo.astype(f32)
    wqs = (g[:, None] * wq).astype(F8)
    wks = (g[:, None] * wk).astype(F8)
    wvs = (g[:, None] * wv).astype(F8)
    wos = wo.astype(F8)
    bq_eff = (bq.astype(f32) + b @ wq).reshape(NCH, 128).T.copy()          # [128, NCH]
    bk_eff = ((bk.astype(f32) + b @ wk) * 0.125).reshape(NCH, 128).T.copy()
    # relpad[j, h*32+q] = rel_bias[h, q, j] for j<32, zero-padded to 128 rows
    relpad = np.zeros((128, H * 32), dtype=f32)
    relpad[:32] = rel_bias.astype(f32).transpose(2, 0, 1).reshape(32, H * 32)
    ident = np.eye(128, dtype=f32).astype(BF16)
    id4pad = np.zeros((128, 128), dtype=f32)
    id4pad[:32] = np.tile(np.eye(32, dtype=f32), (1, 4))
    bdones = np.kron(np.eye(4, dtype=f32), np.ones((32, 32), dtype=f32))
    # bv/bo/ln_b contributions that survive softmax-normalization exactly:
    # out += ((ln_b@wv + bv) @ wo + bo). Zero for this problem's fills.
    c0 = (b @ wv + bv.astype(f32)) @ wo + bo.astype(f32)
    return dict(
        wqs=wqs, wks=wks, wvs=wvs, wos=wos,
        bq_eff=np.ascontiguousarray(bq_eff), bk_eff=np.ascontiguousarray(bk_eff),
        relpad=relpad.astype(BF16), ident=ident,
        id4pad=id4pad.astype(BF16), bdones=bdones.astype(BF16),
    ), c0


_BUILT = {}


def _get_nc(repeat: int = 1):
    if repeat not in _BUILT:
        _BUILT[repeat] = build_nc(repeat)
    return _BUILT[repeat]


def make_in_maps(inputs: dict, consts: dict) -> list:
    x = np.asarray(inputs["x"], dtype=np.float32).reshape(B * S, D)
    in_maps = []
    for c in range(N_CORES):
        m = dict(consts)
        m["x"] = np.ascontiguousarray(x[c * TPC : (c + 1) * TPC])
        in_maps.append(m)
    return in_maps


def kernel(**inputs) -> np.ndarray:
    from concourse.bass_utils import run_bass_kernel_spmd

    consts, c0 = _host_constants(
        inputs["ln_g"], inputs["ln_b"], inputs["wq"], inputs["bq"],
        inputs["wk"], inputs["bk"], inputs["wv"], inputs["bv"],
        inputs["wo"], inputs["bo"], inputs["rel_bias"],
    )
    nc = _get_nc(1)
    in_maps = make_in_maps(inputs, consts)
    res = run_bass_kernel_spmd(nc, in_maps, core_ids=list(range(N_CORES)), trace=False)
    out = np.concatenate([res.results[c]["y"] for c in range(N_CORES)], axis=0)
    out = out.reshape(B, S, D)
    if np.any(c0 != 0.0):
        out = out + c0.astype(np.float32)
    return out

